# revision 1
# baseline (speedup 1.0000x reference)
"""Trainium2 Bass kernel for causal multi-head attention block (GPT-style).

Reference computation (fp32):
    qkv = x @ w_attn + b_attn          # [B,S,3E], heads interleaved per 192 cols
    q,k,v per head (d=64), scores = q k^T / 8, causal mask, softmax
    a = softmax @ v ; h = a @ w_proj + b_proj

Sharding (8 cores): core c -> batch b = c//4, head group g = c%4 (4 heads).
Each core computes qkv for its heads, full causal attention, and a partial
c_proj over its 256 e_in rows; a 4-way ReduceScatter(add) per batch group
yields each core's 512-token chunk of the final output. b_proj added on host.

Device layouts (host pre-marshals everything; fp32 has no DMA transpose):
    xT   [1024, 2048]   x[b]^T (e on partitions)
    wq   [128, 2, 8, 128]  per pair p: cols [qA|qB], PRE-SCALED by 1/8
    wk   [128, 2, 8, 128]  per pair p: cols [kA|kB]
    wv   [128, 8, 256]     4 heads' v cols side by side
    bq   [128, 2]  concat(bq_A,bq_B)/8 ; bk likewise unscaled
    bv   [128, 256]        v bias replicated across partitions
    tri  [128, 128]  upper-tri (key<=query) ; tri2 [128, 256] = [0 | tri]
    wp   [64, 4, 1024]     w_proj rows per head

On-device dataflow per head pair (heads stacked on partition halves):
    QT/KT [128, 2048] = w^T x^T via PE (fp32r), bias via ACT copy
    S^T[key,q] psum = KT_h^T QT_h (K=64, head A rows 0-63, head B 64-127)
    P = exp(S^T) via ACT (no max-sub needed: |scores| < ~3), tri-masked
    a^T|denom psum[65,512] += [V_h|1]^T P  (ones col gives softmax denom)
    at = a^T * recip(denom) broadcast  -> c_proj lhsT [64, tok]
"""

import os
import sys

import numpy as np

if "/opt/trn_rl_repo" not in sys.path:
    sys.path.insert(0, "/opt/trn_rl_repo")

B, S, E, H, D = 2, 2048, 1024, 16, 64
N_CORES = 8
PAIRS = 2  # head pairs per core
ET = 8  # e tiles of 128 over E=1024
QT_N = 4  # query tiles of 512
TT_N = 4  # token tiles of 512 (qkv QK rhs)
VT_N = 16  # token tiles of 128 (V / c_proj)

_cache = {}


def _build():
    import concourse.bass as bass
    import concourse.mybir as mybir
    import concourse.tile as tile
    from concourse import bacc
    from contextlib import ExitStack

    f32 = mybir.dt.float32
    f32r = mybir.dt.float32r
    bf16 = mybir.dt.bfloat16
    ALU = mybir.AluOpType
    AF = mybir.ActivationFunctionType

    nc = bacc.Bacc(
        "TRN2", target_bir_lowering=False, debug=False, num_devices=N_CORES
    )

    xT_d = nc.declare_dram_parameter("xT", [E, S], bf16, isOutput=False)
    wq_d = nc.declare_dram_parameter("wq", [128, PAIRS, ET, 128], bf16, isOutput=False)
    wk_d = nc.declare_dram_parameter("wk", [128, PAIRS, ET, 128], bf16, isOutput=False)
    wv_d = nc.declare_dram_parameter("wv", [128, ET, 256], bf16, isOutput=False)
    bq_d = nc.declare_dram_parameter("bq", [128, PAIRS], f32, isOutput=False)
    bk_d = nc.declare_dram_parameter("bk", [128, PAIRS], f32, isOutput=False)
    bv_d = nc.declare_dram_parameter("bv", [128, 256], f32, isOutput=False)
    tri_d = nc.declare_dram_parameter("tri", [128, 128], bf16, isOutput=False)
    wp_d = nc.declare_dram_parameter("wp", [128, PAIRS, 1024], bf16, isOutput=False)
    sel_d = nc.declare_dram_parameter("sel", [128, PAIRS, 128], bf16, isOutput=False)
    out_d = nc.declare_dram_parameter("out", [512, 1024], f32, isOutput=True)

    with ExitStack() as ctx:
        ctx.enter_context(
            nc.allow_low_precision(reason="fp32r tiles hold full fp32 bits in SBUF")
        )
        tc = ctx.enter_context(tile.TileContext(nc))
        const = ctx.enter_context(tc.tile_pool(name="const", bufs=1))
        dram = ctx.enter_context(tc.tile_pool(name="dram", bufs=1, space="DRAM"))
        psum = ctx.enter_context(tc.tile_pool(name="psum", bufs=4, space="PSUM"))
        psum_av = ctx.enter_context(tc.tile_pool(name="psum_av", bufs=2, space="PSUM"))
        pbuf = ctx.enter_context(tc.tile_pool(name="pbuf", bufs=6))

        # ---- persistent SBUF tensors ----
        xT = const.tile([128, ET, S], bf16, tag="xT")  # 8 MB
        wq = const.tile([128, PAIRS, ET, 128], bf16, tag="wq")
        wk = const.tile([128, PAIRS, ET, 128], bf16, tag="wk")
        wv = const.tile([128, ET, 256], bf16, tag="wv")
        bq = const.tile([128, PAIRS], f32, tag="bq")
        bk = const.tile([128, PAIRS], f32, tag="bk")
        bv = const.tile([128, 256], f32, tag="bv")
        tri = const.tile([128, 128], bf16, tag="tri")
        wp = const.tile([128, PAIRS, 1024], bf16, tag="wp")
        sel = const.tile([128, PAIRS, 128], bf16, tag="sel")
        den4 = const.tile([128, 512], f32, tag="den4")
        nc.vector.memset(den4[:], 1.0)
        qt_sb = const.tile([128, PAIRS, S], bf16, tag="qt")  # rows 0-63 head A
        kt_sb = const.tile([128, PAIRS, S], bf16, tag="kt")
        vv = const.tile([128, VT_N, 4 * 65], bf16, tag="vv")  # [key,tt,(h,d|1)]
        at = const.tile([128, PAIRS, S], bf16, tag="at")  # pair-stacked a^T

        # ---- input DMAs ----
        nc.sync.dma_start(out=wv[:], in_=wv_d[:])
        nc.sync.dma_start(out=wq[:], in_=wq_d[:])
        nc.sync.dma_start(out=wk[:], in_=wk_d[:])
        nc.sync.dma_start(out=bq[:], in_=bq_d[:])
        nc.sync.dma_start(out=bk[:], in_=bk_d[:])
        nc.sync.dma_start(out=bv[:], in_=bv_d[:])
        nc.sync.dma_start(out=tri[:], in_=tri_d[:])
        nc.sync.dma_start(out=wp[:], in_=wp_d[:])
        nc.sync.dma_start(out=sel[:], in_=sel_d[:])
        for et in range(ET):
            for hf in range(2):
                nc.sync.dma_start(
                    out=xT[:, et, hf * 1024 : (hf + 1) * 1024],
                    in_=xT_d[et * 128 : (et + 1) * 128, hf * 1024 : (hf + 1) * 1024],
                )
        nc.vector.memset(vv.rearrange("p t (h e) -> p t h e", h=4)[:, :, :, 64:65], 1.0)

        # ---- Phase A: QKV projections (V first: AV needs it earliest) ----
        for tt in range(VT_N):
            sl = slice(tt * 128, (tt + 1) * 128)
            ps_v = psum.tile([128, 256], f32, tag="mm")
            for et in range(ET):
                nc.tensor.matmul(
                    ps_v,
                    lhsT=xT[:, et, sl],
                    rhs=wv[:, et],
                    start=(et == 0),
                    stop=(et == ET - 1),
                )
            nc.vector.tensor_tensor(
                out=vv.rearrange("p t (h e) -> p t h e", h=4)[:, tt, :, 0:64],
                in0=ps_v.rearrange("p (h e) -> p h e", h=4),
                in1=bv.rearrange("p (h e) -> p h e", h=4),
                op=ALU.add,
            )
        for p in range(PAIRS):
            for tt in range(TT_N):
                sl = slice(tt * 512, (tt + 1) * 512)
                ps_q = psum.tile([128, 512], f32, tag="mm")
                for et in range(ET):
                    nc.tensor.matmul(
                        ps_q,
                        lhsT=wq[:, p, et],
                        rhs=xT[:, et, sl],
                        start=(et == 0),
                        stop=(et == ET - 1),
                    )
                nc.vector.tensor_scalar_add(qt_sb[:, p, sl], ps_q, bq[:, p : p + 1])
                ps_k = psum.tile([128, 512], f32, tag="mm")
                for et in range(ET):
                    nc.tensor.matmul(
                        ps_k,
                        lhsT=wk[:, p, et],
                        rhs=xT[:, et, sl],
                        start=(et == 0),
                        stop=(et == ET - 1),
                    )
                nc.vector.tensor_scalar_add(kt_sb[:, p, sl], ps_k, bk[:, p : p + 1])

        # ---- Phase B+C fused: per query-tile attention -> c_proj -> RS ----
        # c0 for the 4 diagonal key-tiles (j=3 widened to 256 so fp32r stays
        # at full rate; the extra cols are masked by tri2)
        diag_c0 = (0, 128, 256, 384)
        cc_in = []
        cc_out = []
        for qt in range(QT_N):
            cc_in.append(
                dram.tile([512, 1024], f32, tag=f"cc_in{qt}", name=f"cc_in{qt}")
            )
            cc_out.append(
                dram.tile([128, 1024], f32, tag=f"cc_out{qt}", name=f"cc_out{qt}")
            )
        cc_out_l = []
        for half in range(2):
            cc_out_l.append(
                dram.tile([64, 1024], f32, tag=f"cc_outl{half}", name=f"cc_outl{half}")
            )
        def flush(qt, den4, atu):
            """normalize (recip->broadcast->mult), c_proj, ReduceScatter for qt.

            Called one qt later so the reciprocal latency hides behind the
            next query-tile's score/AV stream on the PE."""
            rec4 = pbuf.tile(
                [128, 512], bf16, tag="recb", bufs=2, name=f"rec_{qt}"
            )
            nc.vector.reciprocal(rec4[:], den4[:])
            for pi in range(PAIRS):
                rb = psum.tile([128, 512], f32, tag="cc", bufs=2, name=f"rb_{qt}_{pi}")
                nc.tensor.matmul(
                    rb, lhsT=sel[:, pi, :], rhs=rec4[:], start=True, stop=True
                )
                nc.vector.tensor_tensor(
                    out=at[:, pi, qt * 512 : (qt + 1) * 512],
                    in0=atu[pi][:],
                    in1=rb[:],
                    op=ALU.mult,
                )
            for tt in range(4 * qt, 4 * qt + 4):
                for nt in range(2):
                    ps_c = psum.tile([128, 512], f32, tag="cc", bufs=2)
                    for pi in range(PAIRS):
                        nc.tensor.matmul(
                            ps_c,
                            lhsT=at[:, pi, tt * 128 : (tt + 1) * 128],
                            rhs=wp[:, pi, nt * 512 : (nt + 1) * 512],
                            start=(pi == 0),
                            stop=(pi == PAIRS - 1),
                        )
                    cst = pbuf.tile([128, 512], f32, tag="cstage", bufs=2, name=f"cst_{tt}_{nt}")
                    nc.scalar.copy(cst[:], ps_c[:])
                    nc.sync.dma_start(
                        out=cc_in[qt][
                            (tt - 4 * qt) * 128 : (tt - 4 * qt + 1) * 128,
                            nt * 512 : (nt + 1) * 512,
                        ],
                        in_=cst[:],
                    )
                # split the LAST query tile's ReduceScatter in two to shorten
                # the serial tail
                if qt == QT_N - 1 and tt in (4 * qt + 1, 4 * qt + 3):
                    half = 0 if tt == 4 * qt + 1 else 1
                    nc.gpsimd.collective_compute(
                        "ReduceScatter",
                        mybir.AluOpType.add,
                        replica_groups=[[0, 1, 2, 3], [4, 5, 6, 7]],
                        ins=[cc_in[qt][half * 256 : (half + 1) * 256, :].opt()],
                        outs=[cc_out_l[half][:].opt()],
                    )
                    nc.sync.dma_start(
                        out=out_d[qt * 128 + half * 64 : qt * 128 + (half + 1) * 64, :],
                        in_=cc_out_l[half][:],
                    )
            if qt < QT_N - 1:
                nc.gpsimd.collective_compute(
                    "ReduceScatter",
                    mybir.AluOpType.add,
                    replica_groups=[[0, 1, 2, 3], [4, 5, 6, 7]],
                    ins=[cc_in[qt][:].opt()],
                    outs=[cc_out[qt][:].opt()],
                )
                nc.sync.dma_start(
                    out=out_d[qt * 128 : (qt + 1) * 128, :], in_=cc_out[qt][:]
                )

        pending = None
        for qt in range(QT_N):
            if pending is not None:
                flush(*pending)
                pending = None
            den4 = pbuf.tile([128, 512], f32, tag="den", bufs=2, name=f"den_{qt}")
            nc.vector.memset(den4[:], 1.0)
            atu_pair = []
            for p in range(PAIRS):
                av = []
                for hh in range(2):
                    av.append(
                        psum_av.tile([65, 512], f32, tag="av", name=f"av_{p}_{qt}_{hh}")
                    )
                nkt = 4 * qt + 4
                for kt in range(nkt):
                    j = kt - 4 * qt
                    c0 = diag_c0[j] if j >= 0 else 0
                    n = 512 - c0
                    ps_s = []
                    pts = []
                    for hh in range(2):
                        base = hh * 64
                        ps_s.append(psum.tile([128, 512], f32, tag="mm",
                                              name=f"ps_s_{p}_{qt}_{kt}_{hh}"))
                        nc.tensor.matmul(
                            ps_s[hh][:, 0:n],
                            lhsT=kt_sb[base : base + 64, p, kt * 128 : (kt + 1) * 128],
                            rhs=qt_sb[base : base + 64, p, qt * 512 + c0 : (qt + 1) * 512],
                            start=True,
                            stop=True,
                        )
                    for hh in range(2):
                        pt = pbuf.tile([128, 512], bf16, tag="p", bufs=8,
                                       name=f"pt_{p}_{qt}_{kt}_{hh}")
                        pts.append(pt)
                        nc.scalar.activation(pt[:, 0:n], ps_s[hh][:, 0:n], AF.Exp)
                        if j >= 0:
                            nc.vector.tensor_tensor(
                                out=pt[:, 0:128], in0=pt[:, 0:128], in1=tri[:],
                                op=ALU.mult,
                            )
                    for hh in range(2):
                        h_idx = 2 * p + hh
                        nc.tensor.matmul(
                            av[hh][:, c0:512],
                            lhsT=vv[:, kt, h_idx * 65 : (h_idx + 1) * 65],
                            rhs=pts[hh][:, 0:n],
                            start=(kt == 0),
                            stop=(kt == nkt - 1),
                        )
                for hh in range(2):
                    h_idx = 2 * p + hh
                    nc.vector.tensor_copy(
                        out=den4[h_idx * 32 : h_idx * 32 + 1, :], in_=av[hh][64:65, :]
                    )
                atu2 = pbuf.tile([128, 512], f32, tag="atu", bufs=4,
                                 name=f"atu_{p}_{qt}")
                nc.vector.tensor_copy(out=atu2[0:64, :], in_=av[0][0:64, :])
                nc.vector.tensor_copy(out=atu2[64:128, :], in_=av[1][0:64, :])
                atu_pair.append(atu2)
            pending = (qt, den4, atu_pair)
        flush(*pending)

    nc.compile()
    return nc


def _prepare_in_maps(x, w_attn, b_attn, w_proj):
    import ml_dtypes

    bf = ml_dtypes.bfloat16
    in_maps = []
    tri = np.triu(np.ones((128, 128), dtype=bf))
    for core in range(N_CORES):
        b, g = core // 4, core % 4
        heads = [4 * g + i for i in range(4)]
        xT = np.ascontiguousarray(x[b].T)  # [1024, 2048]
        wq_blocks, wk_blocks, bq_cols, bk_cols = [], [], [], []
        for pr in range(PAIRS):
            hA, hB = heads[2 * pr], heads[2 * pr + 1]
            wq_blk = np.concatenate(
                [w_attn[:, hA * 192 : hA * 192 + 64], w_attn[:, hB * 192 : hB * 192 + 64]],
                axis=1,
            ) * 0.125
            wk_blk = np.concatenate(
                [
                    w_attn[:, hA * 192 + 64 : hA * 192 + 128],
                    w_attn[:, hB * 192 + 64 : hB * 192 + 128],
                ],
                axis=1,
            )
            # [1024,128] -> [128part, 8et, 128]
            wq_blocks.append(wq_blk.reshape(ET, 128, 128).transpose(1, 0, 2))
            wk_blocks.append(wk_blk.reshape(ET, 128, 128).transpose(1, 0, 2))
            bq_cols.append(
                np.concatenate(
                    [b_attn[hA * 192 : hA * 192 + 64], b_attn[hB * 192 : hB * 192 + 64]]
                ) * 0.125
            )
            bk_cols.append(
                np.concatenate(
                    [
                        b_attn[hA * 192 + 64 : hA * 192 + 128],
                        b_attn[hB * 192 + 64 : hB * 192 + 128],
                    ]
                )
            )
        wq_h = np.stack(wq_blocks, axis=1)  # [128, 2, 8, 128]
        wk_h = np.stack(wk_blocks, axis=1)
        wv_blk = np.concatenate(
            [w_attn[:, h * 192 + 128 : h * 192 + 192] for h in heads], axis=1
        )  # [1024, 256]
        wv_h = wv_blk.reshape(ET, 128, 256).transpose(1, 0, 2)  # [128, 8, 256]
        bv_row = np.concatenate(
            [b_attn[h * 192 + 128 : h * 192 + 192] for h in heads]
        )  # [256]
        bv_h = np.broadcast_to(bv_row, (128, 256)).copy()
        wp_h = np.empty((128, PAIRS, 1024), dtype=np.float32)
        sel_h = np.zeros((128, PAIRS, 128), dtype=np.float32)
        for pr in range(PAIRS):
            hA, hB = heads[2 * pr], heads[2 * pr + 1]
            wp_h[0:64, pr, :] = w_proj[hA * 64 : (hA + 1) * 64, :]
            wp_h[64:128, pr, :] = w_proj[hB * 64 : (hB + 1) * 64, :]
            sel_h[(2 * pr) * 32, pr, 0:64] = 1.0
            sel_h[(2 * pr + 1) * 32, pr, 64:128] = 1.0
        in_maps.append(
            {
                "xT": np.ascontiguousarray(xT.astype(bf)),
                "wq": np.ascontiguousarray(wq_h.astype(bf)),
                "wk": np.ascontiguousarray(wk_h.astype(bf)),
                "wv": np.ascontiguousarray(wv_h.astype(bf)),
                "bq": np.ascontiguousarray(np.stack(bq_cols, 1), dtype=np.float32),
                "bk": np.ascontiguousarray(np.stack(bk_cols, 1), dtype=np.float32),
                "bv": bv_h.astype(np.float32),
                "tri": tri,
                "wp": np.ascontiguousarray(wp_h.astype(bf)),
                "sel": np.ascontiguousarray(sel_h.astype(bf)),
            }
        )
    return in_maps


def _run(x, w_attn, b_attn, w_proj, b_proj, trace=False):
    from concourse.bass_utils import run_bass_kernel_spmd

    if "nc" not in _cache:
        _cache["nc"] = _build()
    nc = _cache["nc"]
    in_maps = _prepare_in_maps(x, w_attn, b_attn, w_proj)
    res = run_bass_kernel_spmd(nc, in_maps, list(range(N_CORES)), trace=trace)
    outs = []
    for b in range(B):
        full = np.empty((S, E), dtype=np.float32)
        for r_ in range(4):
            core_out = res.results[4 * b + r_]["out"]
            for qt in range(QT_N - 1):
                full[qt * 512 + r_ * 128 : qt * 512 + (r_ + 1) * 128] = core_out[
                    qt * 128 : (qt + 1) * 128
                ]
            # last query tile was reduce-scattered in two 256-token halves
            for half in range(2):
                t0 = (QT_N - 1) * 512 + half * 256
                full[t0 + r_ * 64 : t0 + (r_ + 1) * 64] = core_out[
                    (QT_N - 1) * 128 + half * 64 : (QT_N - 1) * 128 + (half + 1) * 64
                ]
        outs.append(full + b_proj[None, :])
    return np.stack(outs).astype(np.float32), res


def kernel(x, w_attn, b_attn, w_proj, b_proj):
    x = np.asarray(x, dtype=np.float32)
    w_attn = np.asarray(w_attn, dtype=np.float32)
    b_attn = np.asarray(b_attn, dtype=np.float32)
    w_proj = np.asarray(w_proj, dtype=np.float32)
    b_proj = np.asarray(b_proj, dtype=np.float32)
    out, _ = _run(x, w_attn, b_attn, w_proj, b_proj, trace=False)
    return out



# revision 5
# speedup vs baseline: 1.1260x; 1.1260x over previous
"""Trainium2 Bass kernel for causal multi-head attention block (GPT-style).

Reference computation (fp32):
    qkv = x @ w_attn + b_attn          # [B,S,3E], heads interleaved per 192 cols
    q,k,v per head (d=64), scores = q k^T / 8, causal mask, softmax
    a = softmax @ v ; h = a @ w_proj + b_proj

Sharding (8 cores): core c -> batch b = c//4, head group g = c%4 (4 heads).
Each core computes qkv for its heads, full causal attention, and a partial
c_proj over its 256 e_in rows; a 4-way ReduceScatter(add) per batch group
yields each core's 512-token chunk of the final output. b_proj added on host.

Device layouts (host pre-marshals everything; fp32 has no DMA transpose):
    xT   [1024, 2048]   x[b]^T (e on partitions)
    wq   [128, 2, 8, 128]  per pair p: cols [qA|qB], PRE-SCALED by 1/8
    wk   [128, 2, 8, 128]  per pair p: cols [kA|kB]
    wv   [128, 8, 256]     4 heads' v cols side by side
    bq   [128, 2]  concat(bq_A,bq_B)/8 ; bk likewise unscaled
    bv   [128, 256]        v bias replicated across partitions
    tri  [128, 128]  upper-tri (key<=query) ; tri2 [128, 256] = [0 | tri]
    wp   [64, 4, 1024]     w_proj rows per head

On-device dataflow per head pair (heads stacked on partition halves):
    QT/KT [128, 2048] = w^T x^T via PE (fp32r), bias via ACT copy
    S^T[key,q] psum = KT_h^T QT_h (K=64, head A rows 0-63, head B 64-127)
    P = exp(S^T) via ACT (no max-sub needed: |scores| < ~3), tri-masked
    a^T|denom psum[65,512] += [V_h|1]^T P  (ones col gives softmax denom)
    at = a^T * recip(denom) broadcast  -> c_proj lhsT [64, tok]
"""

import os
import sys

import numpy as np

if "/opt/trn_rl_repo" not in sys.path:
    sys.path.insert(0, "/opt/trn_rl_repo")

B, S, E, H, D = 2, 2048, 1024, 16, 64
N_CORES = 8
PAIRS = 2  # head pairs per core
ET = 8  # e tiles of 128 over E=1024
QT_N = 4  # query tiles of 512
TT_N = 4  # token tiles of 512 (qkv QK rhs)
VT_N = 16  # token tiles of 128 (V / c_proj)

_cache = {}


def _build():
    import concourse.bass as bass
    import concourse.mybir as mybir
    import concourse.tile as tile
    from concourse import bacc
    from contextlib import ExitStack

    f32 = mybir.dt.float32
    f32r = mybir.dt.float32r
    bf16 = mybir.dt.bfloat16
    ALU = mybir.AluOpType
    AF = mybir.ActivationFunctionType

    nc = bacc.Bacc(
        "TRN2", target_bir_lowering=False, debug=False, num_devices=N_CORES
    )

    xT_d = nc.declare_dram_parameter("xT", [E, S], bf16, isOutput=False)
    wq_d = nc.declare_dram_parameter("wq", [128, PAIRS, ET, 128], bf16, isOutput=False)
    wk_d = nc.declare_dram_parameter("wk", [128, PAIRS, ET, 128], bf16, isOutput=False)
    wv_d = nc.declare_dram_parameter("wv", [128, ET, 256], bf16, isOutput=False)
    bq_d = nc.declare_dram_parameter("bq", [128, PAIRS], f32, isOutput=False)
    bk_d = nc.declare_dram_parameter("bk", [128, PAIRS], f32, isOutput=False)
    bv_d = nc.declare_dram_parameter("bv", [128, 256], f32, isOutput=False)
    tri_d = nc.declare_dram_parameter("tri", [128, 128], bf16, isOutput=False)
    wp_d = nc.declare_dram_parameter("wp", [128, PAIRS, 1024], bf16, isOutput=False)
    sel_d = nc.declare_dram_parameter("sel", [128, PAIRS, 128], bf16, isOutput=False)
    out_d = nc.declare_dram_parameter("out", [512, 1024], bf16, isOutput=True)

    with ExitStack() as ctx:
        ctx.enter_context(
            nc.allow_low_precision(reason="fp32r tiles hold full fp32 bits in SBUF")
        )
        tc = ctx.enter_context(tile.TileContext(nc))
        const = ctx.enter_context(tc.tile_pool(name="const", bufs=1))
        dram = ctx.enter_context(tc.tile_pool(name="dram", bufs=1, space="DRAM"))
        psum = ctx.enter_context(tc.tile_pool(name="psum", bufs=4, space="PSUM"))
        psum_av = ctx.enter_context(tc.tile_pool(name="psum_av", bufs=2, space="PSUM"))
        pbuf = ctx.enter_context(tc.tile_pool(name="pbuf", bufs=6))

        # ---- persistent SBUF tensors ----
        xT = const.tile([128, ET, S], bf16, tag="xT")  # 8 MB
        wq = const.tile([128, PAIRS, ET, 128], bf16, tag="wq")
        wk = const.tile([128, PAIRS, ET, 128], bf16, tag="wk")
        wv = const.tile([128, ET, 256], bf16, tag="wv")
        bq = const.tile([128, PAIRS], f32, tag="bq")
        bk = const.tile([128, PAIRS], f32, tag="bk")
        bv = const.tile([128, 256], f32, tag="bv")
        tri = const.tile([128, 128], bf16, tag="tri")
        wp = const.tile([128, PAIRS, 1024], bf16, tag="wp")
        sel = const.tile([128, PAIRS, 128], bf16, tag="sel")
        den4 = const.tile([128, 512], f32, tag="den4")
        nc.vector.memset(den4[:], 1.0)
        qt_sb = const.tile([128, PAIRS, S], bf16, tag="qt")  # rows 0-63 head A
        kt_sb = const.tile([128, PAIRS, S], bf16, tag="kt")
        vv = const.tile([128, VT_N, 4 * 65], bf16, tag="vv")  # [key,tt,(h,d|1)]
        at = const.tile([128, PAIRS, S], bf16, tag="at")  # pair-stacked a^T

        # ---- input DMAs (V path first; xT half 0 before half 1 so the
        # first V/Q/K matmuls can start while the rest streams in) ----
        nc.sync.dma_start(out=wv[:], in_=wv_d[:])
        nc.sync.dma_start(out=bv[:], in_=bv_d[:])
        nc.sync.dma_start(out=wq[:], in_=wq_d[:])
        nc.sync.dma_start(out=wk[:], in_=wk_d[:])
        nc.sync.dma_start(out=bq[:], in_=bq_d[:])
        nc.sync.dma_start(out=bk[:], in_=bk_d[:])
        nc.sync.dma_start(out=tri[:], in_=tri_d[:])
        nc.sync.dma_start(out=wp[:], in_=wp_d[:])
        nc.sync.dma_start(out=sel[:], in_=sel_d[:])
        for hf in range(2):
            for et in range(ET):
                nc.sync.dma_start(
                    out=xT[:, et, hf * 1024 : (hf + 1) * 1024],
                    in_=xT_d[et * 128 : (et + 1) * 128, hf * 1024 : (hf + 1) * 1024],
                )
        nc.vector.memset(vv.rearrange("p t (h e) -> p t h e", h=4)[:, :, :, 64:65], 1.0)

        # ---- Phase A: QKV projections (V first: AV needs it earliest) ----
        for tt in range(VT_N):
            sl = slice(tt * 128, (tt + 1) * 128)
            ps_v = psum.tile([128, 256], f32, tag="mm")
            for et in range(ET):
                nc.tensor.matmul(
                    ps_v,
                    lhsT=xT[:, et, sl],
                    rhs=wv[:, et],
                    start=(et == 0),
                    stop=(et == ET - 1),
                )
            nc.vector.tensor_tensor(
                out=vv.rearrange("p t (h e) -> p t h e", h=4)[:, tt, :, 0:64],
                in0=ps_v.rearrange("p (h e) -> p h e", h=4),
                in1=bv.rearrange("p (h e) -> p h e", h=4),
                op=ALU.add,
            )
        for p in range(PAIRS):
            for tt in range(TT_N):
                sl = slice(tt * 512, (tt + 1) * 512)
                ps_q = psum.tile([128, 512], f32, tag="mm")
                for et in range(ET):
                    nc.tensor.matmul(
                        ps_q,
                        lhsT=wq[:, p, et],
                        rhs=xT[:, et, sl],
                        start=(et == 0),
                        stop=(et == ET - 1),
                    )
                nc.vector.tensor_scalar_add(qt_sb[:, p, sl], ps_q, bq[:, p : p + 1])
                ps_k = psum.tile([128, 512], f32, tag="mm")
                for et in range(ET):
                    nc.tensor.matmul(
                        ps_k,
                        lhsT=wk[:, p, et],
                        rhs=xT[:, et, sl],
                        start=(et == 0),
                        stop=(et == ET - 1),
                    )
                nc.vector.tensor_scalar_add(kt_sb[:, p, sl], ps_k, bk[:, p : p + 1])

        # ---- Phase B+C fused: per query-tile attention -> c_proj -> RS ----
        # c0 for the 4 diagonal key-tiles (j=3 widened to 256 so fp32r stays
        # at full rate; the extra cols are masked by tri2)
        diag_c0 = (0, 128, 256, 384)
        cc_in = []
        cc_out = []
        for qt in range(QT_N):
            cc_in.append(
                dram.tile([512, 1024], bf16, tag=f"cc_in{qt}", name=f"cc_in{qt}")
            )
            cc_out.append(
                dram.tile([128, 1024], bf16, tag=f"cc_out{qt}", name=f"cc_out{qt}")
            )
        cc_out_l = []
        for half in range(2):
            cc_out_l.append(
                dram.tile([64, 1024], bf16, tag=f"cc_outl{half}", name=f"cc_outl{half}")
            )
        def flush(qt, den4, atu):
            """normalize (recip->broadcast->mult), c_proj, ReduceScatter for qt.

            Called one qt later so the reciprocal latency hides behind the
            next query-tile's score/AV stream on the PE."""
            rec4 = pbuf.tile(
                [128, 512], bf16, tag="recb", bufs=2, name=f"rec_{qt}"
            )
            nc.vector.reciprocal(rec4[:], den4[:])
            for pi in range(PAIRS):
                rb = psum.tile([128, 512], f32, tag="cc", bufs=2, name=f"rb_{qt}_{pi}")
                nc.tensor.matmul(
                    rb, lhsT=sel[:, pi, :], rhs=rec4[:], start=True, stop=True
                )
                nc.vector.tensor_tensor(
                    out=at[:, pi, qt * 512 : (qt + 1) * 512],
                    in0=atu[pi][:],
                    in1=rb[:],
                    op=ALU.mult,
                )
            for tt in range(4 * qt, 4 * qt + 4):
                for nt in range(2):
                    ps_c = psum.tile([128, 512], f32, tag="cc", bufs=2)
                    for pi in range(PAIRS):
                        nc.tensor.matmul(
                            ps_c,
                            lhsT=at[:, pi, tt * 128 : (tt + 1) * 128],
                            rhs=wp[:, pi, nt * 512 : (nt + 1) * 512],
                            start=(pi == 0),
                            stop=(pi == PAIRS - 1),
                        )
                    cst = pbuf.tile([128, 512], bf16, tag="cstage", bufs=4, name=f"cst_{tt}_{nt}")
                    nc.vector.tensor_copy(out=cst[:], in_=ps_c[:])
                    nc.sync.dma_start(
                        out=cc_in[qt][
                            (tt - 4 * qt) * 128 : (tt - 4 * qt + 1) * 128,
                            nt * 512 : (nt + 1) * 512,
                        ],
                        in_=cst[:],
                    )
                # split the LAST query tile's ReduceScatter in two to shorten
                # the serial tail
                if qt == QT_N - 1 and tt in (4 * qt + 1, 4 * qt + 3):
                    half = 0 if tt == 4 * qt + 1 else 1
                    nc.gpsimd.collective_compute(
                        "ReduceScatter",
                        mybir.AluOpType.add,
                        replica_groups=[[0, 1, 2, 3], [4, 5, 6, 7]],
                        ins=[cc_in[qt][half * 256 : (half + 1) * 256, :].opt()],
                        outs=[cc_out_l[half][:].opt()],
                    )
                    nc.sync.dma_start(
                        out=out_d[qt * 128 + half * 64 : qt * 128 + (half + 1) * 64, :],
                        in_=cc_out_l[half][:],
                    )
            if qt < QT_N - 1:
                nc.gpsimd.collective_compute(
                    "ReduceScatter",
                    mybir.AluOpType.add,
                    replica_groups=[[0, 1, 2, 3], [4, 5, 6, 7]],
                    ins=[cc_in[qt][:].opt()],
                    outs=[cc_out[qt][:].opt()],
                )
                nc.sync.dma_start(
                    out=out_d[qt * 128 : (qt + 1) * 128, :], in_=cc_out[qt][:]
                )

        pending = None
        for qt in range(QT_N):
            if pending is not None:
                flush(*pending)
                pending = None
            den4 = pbuf.tile([128, 512], f32, tag="den", bufs=2, name=f"den_{qt}")
            nc.vector.memset(den4[:], 1.0)
            atu_pair = []
            for p in range(PAIRS):
                av = []
                for hh in range(2):
                    av.append(
                        psum_av.tile([65, 512], f32, tag="av", name=f"av_{p}_{qt}_{hh}")
                    )
                nkt = 4 * qt + 4
                for kt in range(nkt):
                    j = kt - 4 * qt
                    c0 = diag_c0[j] if j >= 0 else 0
                    n = 512 - c0
                    ps_s = []
                    pts = []
                    for hh in range(2):
                        base = hh * 64
                        ps_s.append(psum.tile([128, 512], f32, tag="mm",
                                              name=f"ps_s_{p}_{qt}_{kt}_{hh}"))
                        nc.tensor.matmul(
                            ps_s[hh][:, 0:n],
                            lhsT=kt_sb[base : base + 64, p, kt * 128 : (kt + 1) * 128],
                            rhs=qt_sb[base : base + 64, p, qt * 512 + c0 : (qt + 1) * 512],
                            start=True,
                            stop=True,
                        )
                    for hh in range(2):
                        pt = pbuf.tile([128, 512], bf16, tag="p", bufs=8,
                                       name=f"pt_{p}_{qt}_{kt}_{hh}")
                        pts.append(pt)
                        nc.scalar.activation(pt[:, 0:n], ps_s[hh][:, 0:n], AF.Exp)
                        if j >= 0:
                            nc.vector.tensor_tensor(
                                out=pt[:, 0:128], in0=pt[:, 0:128], in1=tri[:],
                                op=ALU.mult,
                            )
                    for hh in range(2):
                        h_idx = 2 * p + hh
                        nc.tensor.matmul(
                            av[hh][:, c0:512],
                            lhsT=vv[:, kt, h_idx * 65 : (h_idx + 1) * 65],
                            rhs=pts[hh][:, 0:n],
                            start=(kt == 0),
                            stop=(kt == nkt - 1),
                        )
                for hh in range(2):
                    h_idx = 2 * p + hh
                    nc.vector.tensor_copy(
                        out=den4[h_idx * 32 : h_idx * 32 + 1, :], in_=av[hh][64:65, :]
                    )
                atu2 = pbuf.tile([128, 512], f32, tag="atu", bufs=4,
                                 name=f"atu_{p}_{qt}")
                nc.vector.tensor_copy(out=atu2[0:64, :], in_=av[0][0:64, :])
                nc.vector.tensor_copy(out=atu2[64:128, :], in_=av[1][0:64, :])
                atu_pair.append(atu2)
            pending = (qt, den4, atu_pair)
        flush(*pending)

    nc.compile()
    return nc


def _prepare_in_maps(x, w_attn, b_attn, w_proj):
    import ml_dtypes

    bf = ml_dtypes.bfloat16
    in_maps = []
    tri = np.triu(np.ones((128, 128), dtype=bf))
    for core in range(N_CORES):
        b, g = core // 4, core % 4
        heads = [4 * g + i for i in range(4)]
        xT = np.ascontiguousarray(x[b].T)  # [1024, 2048]
        wq_blocks, wk_blocks, bq_cols, bk_cols = [], [], [], []
        for pr in range(PAIRS):
            hA, hB = heads[2 * pr], heads[2 * pr + 1]
            wq_blk = np.concatenate(
                [w_attn[:, hA * 192 : hA * 192 + 64], w_attn[:, hB * 192 : hB * 192 + 64]],
                axis=1,
            ) * 0.125
            wk_blk = np.concatenate(
                [
                    w_attn[:, hA * 192 + 64 : hA * 192 + 128],
                    w_attn[:, hB * 192 + 64 : hB * 192 + 128],
                ],
                axis=1,
            )
            # [1024,128] -> [128part, 8et, 128]
            wq_blocks.append(wq_blk.reshape(ET, 128, 128).transpose(1, 0, 2))
            wk_blocks.append(wk_blk.reshape(ET, 128, 128).transpose(1, 0, 2))
            bq_cols.append(
                np.concatenate(
                    [b_attn[hA * 192 : hA * 192 + 64], b_attn[hB * 192 : hB * 192 + 64]]
                ) * 0.125
            )
            bk_cols.append(
                np.concatenate(
                    [
                        b_attn[hA * 192 + 64 : hA * 192 + 128],
                        b_attn[hB * 192 + 64 : hB * 192 + 128],
                    ]
                )
            )
        wq_h = np.stack(wq_blocks, axis=1)  # [128, 2, 8, 128]
        wk_h = np.stack(wk_blocks, axis=1)
        wv_blk = np.concatenate(
            [w_attn[:, h * 192 + 128 : h * 192 + 192] for h in heads], axis=1
        )  # [1024, 256]
        wv_h = wv_blk.reshape(ET, 128, 256).transpose(1, 0, 2)  # [128, 8, 256]
        bv_row = np.concatenate(
            [b_attn[h * 192 + 128 : h * 192 + 192] for h in heads]
        )  # [256]
        bv_h = np.broadcast_to(bv_row, (128, 256)).copy()
        wp_h = np.empty((128, PAIRS, 1024), dtype=np.float32)
        sel_h = np.zeros((128, PAIRS, 128), dtype=np.float32)
        for pr in range(PAIRS):
            hA, hB = heads[2 * pr], heads[2 * pr + 1]
            wp_h[0:64, pr, :] = w_proj[hA * 64 : (hA + 1) * 64, :]
            wp_h[64:128, pr, :] = w_proj[hB * 64 : (hB + 1) * 64, :]
            sel_h[(2 * pr) * 32, pr, 0:64] = 1.0
            sel_h[(2 * pr + 1) * 32, pr, 64:128] = 1.0
        in_maps.append(
            {
                "xT": np.ascontiguousarray(xT.astype(bf)),
                "wq": np.ascontiguousarray(wq_h.astype(bf)),
                "wk": np.ascontiguousarray(wk_h.astype(bf)),
                "wv": np.ascontiguousarray(wv_h.astype(bf)),
                "bq": np.ascontiguousarray(np.stack(bq_cols, 1), dtype=np.float32),
                "bk": np.ascontiguousarray(np.stack(bk_cols, 1), dtype=np.float32),
                "bv": bv_h.astype(np.float32),
                "tri": tri,
                "wp": np.ascontiguousarray(wp_h.astype(bf)),
                "sel": np.ascontiguousarray(sel_h.astype(bf)),
            }
        )
    return in_maps


def _run(x, w_attn, b_attn, w_proj, b_proj, trace=False):
    from concourse.bass_utils import run_bass_kernel_spmd

    if "nc" not in _cache:
        _cache["nc"] = _build()
    nc = _cache["nc"]
    in_maps = _prepare_in_maps(x, w_attn, b_attn, w_proj)
    res = run_bass_kernel_spmd(nc, in_maps, list(range(N_CORES)), trace=trace)
    outs = []
    for b in range(B):
        full = np.empty((S, E), dtype=np.float32)
        for r_ in range(4):
            core_out = res.results[4 * b + r_]["out"]
            for qt in range(QT_N - 1):
                full[qt * 512 + r_ * 128 : qt * 512 + (r_ + 1) * 128] = core_out[
                    qt * 128 : (qt + 1) * 128
                ]
            # last query tile was reduce-scattered in two 256-token halves
            for half in range(2):
                t0 = (QT_N - 1) * 512 + half * 256
                full[t0 + r_ * 64 : t0 + (r_ + 1) * 64] = core_out[
                    (QT_N - 1) * 128 + half * 64 : (QT_N - 1) * 128 + (half + 1) * 64
                ]
        outs.append(full + b_proj[None, :])
    return np.stack(outs).astype(np.float32), res


def kernel(x, w_attn, b_attn, w_proj, b_proj):
    x = np.asarray(x, dtype=np.float32)
    w_attn = np.asarray(w_attn, dtype=np.float32)
    b_attn = np.asarray(b_attn, dtype=np.float32)
    w_proj = np.asarray(w_proj, dtype=np.float32)
    b_proj = np.asarray(b_proj, dtype=np.float32)
    out, _ = _run(x, w_attn, b_attn, w_proj, b_proj, trace=False)
    return out



# revision 6
# speedup vs baseline: 1.3755x; 1.2216x over previous
"""Trainium2 Bass kernel for causal multi-head attention block (GPT-style).

Reference computation (fp32):
    qkv = x @ w_attn + b_attn          # [B,S,3E], heads interleaved per 192 cols
    q,k,v per head (d=64), scores = q k^T / 8, causal mask, softmax
    a = softmax @ v ; h = a @ w_proj + b_proj

Sharding (8 cores): core c -> batch b = c//4, head group g = c%4 (4 heads).
Each core computes qkv for its heads, full causal attention, and a partial
c_proj over its 256 e_in rows; a 4-way ReduceScatter(add) per batch group
yields each core's 512-token chunk of the final output (bf16 on the wire).
b_proj added on host.

Device layouts (host pre-marshals everything; fp32 has no DMA transpose):
    xT   [1024, 2048]   x[b]^T (e on partitions)
    wq   [128, 2, 8, 128]  per pair p: cols [qA|qB], PRE-SCALED by 1/8
    wk   [128, 2, 8, 128]  per pair p: cols [kA|kB]
    wv   [128, 8, 256]     4 heads' v cols side by side
    bq   [128, 2]  concat(bq_A,bq_B)/8 ; bk likewise unscaled
    bv   [128, 256]        v bias replicated across partitions
    tri  [128, 128]  upper-tri (key<=query)
    wp   [64, 4, 1024]     w_proj rows per head

On-device dataflow per head pair (heads stacked on partition halves):
    QT/KT [128, 2048] = w^T x^T via PE, bias via DVE
    S^T[key,q] psum [128,1024] = both heads' scores side by side
    P = exp(S^T) via one ACT instr per kt (off-diag), tri-masked on diag
    a^T|denom psum[65,512] += [V_h|1]^T P  (ones col gives softmax denom)
    at = a^T * recip(denom) broadcast  -> c_proj lhsT [64, tok]

Pipelining: two tiny warm-up AllReduces absorb core launch skew during the
QKV phase so the real per-qt ReduceScatters run at steady-state cost; the
c_proj + staging of query-tile qt-1 is interleaved into the first 8 kt
iterations of qt's attention so the PE/ACT streams never drain.
"""

import os
import sys

import numpy as np

if "/opt/trn_rl_repo" not in sys.path:
    sys.path.insert(0, "/opt/trn_rl_repo")

B, S, E, H, D = 2, 2048, 1024, 16, 64
N_CORES = 8
PAIRS = 2  # head pairs per core
ET = 8  # e tiles of 128 over E=1024
QT_N = 4  # query tiles of 512
TT_N = 4  # token tiles of 512 (qkv QK rhs)
VT_N = 16  # token tiles of 128 (V / c_proj)

_cache = {}


def _build():
    import concourse.bass as bass
    import concourse.mybir as mybir
    import concourse.tile as tile
    from concourse import bacc
    from contextlib import ExitStack

    f32 = mybir.dt.float32
    bf16 = mybir.dt.bfloat16
    ALU = mybir.AluOpType
    AF = mybir.ActivationFunctionType

    nc = bacc.Bacc(
        "TRN2", target_bir_lowering=False, debug=False, num_devices=N_CORES
    )

    xT_d = nc.declare_dram_parameter("xT", [E, S], bf16, isOutput=False)
    wq_d = nc.declare_dram_parameter("wq", [128, PAIRS, ET, 128], bf16, isOutput=False)
    wk_d = nc.declare_dram_parameter("wk", [128, PAIRS, ET, 128], bf16, isOutput=False)
    wv_d = nc.declare_dram_parameter("wv", [128, ET, 256], bf16, isOutput=False)
    bq_d = nc.declare_dram_parameter("bq", [128, PAIRS], f32, isOutput=False)
    bk_d = nc.declare_dram_parameter("bk", [128, PAIRS], f32, isOutput=False)
    bv_d = nc.declare_dram_parameter("bv", [128, 256], f32, isOutput=False)
    tri_d = nc.declare_dram_parameter("tri", [128, 128], bf16, isOutput=False)
    wp_d = nc.declare_dram_parameter("wp", [128, PAIRS, 1024], bf16, isOutput=False)
    sel_d = nc.declare_dram_parameter("sel", [128, PAIRS, 128], bf16, isOutput=False)
    out_d = nc.declare_dram_parameter("out", [512, 1024], bf16, isOutput=True)

    with ExitStack() as ctx:
        ctx.enter_context(
            nc.allow_low_precision(reason="bf16 matmuls/collectives within 2e-2 tol")
        )
        tc = ctx.enter_context(tile.TileContext(nc))
        const = ctx.enter_context(tc.tile_pool(name="const", bufs=1))
        dram = ctx.enter_context(tc.tile_pool(name="dram", bufs=1, space="DRAM"))
        psum_sc = ctx.enter_context(tc.tile_pool(name="psum_sc", bufs=2, space="PSUM"))
        psum_av = ctx.enter_context(tc.tile_pool(name="psum_av", bufs=2, space="PSUM"))
        psum_cc = ctx.enter_context(tc.tile_pool(name="psum_cc", bufs=2, space="PSUM"))
        pbuf = ctx.enter_context(tc.tile_pool(name="pbuf", bufs=6))

        # ---- persistent SBUF tensors ----
        xT = const.tile([128, ET, S], bf16, tag="xT")  # 4 MB
        wq = const.tile([128, PAIRS, ET, 128], bf16, tag="wq")
        wk = const.tile([128, PAIRS, ET, 128], bf16, tag="wk")
        wv = const.tile([128, ET, 256], bf16, tag="wv")
        bq = const.tile([128, PAIRS], f32, tag="bq")
        bk = const.tile([128, PAIRS], f32, tag="bk")
        bv = const.tile([128, 256], f32, tag="bv")
        tri = const.tile([128, 128], bf16, tag="tri")
        wp = const.tile([128, PAIRS, 1024], bf16, tag="wp")
        sel = const.tile([128, PAIRS, 128], bf16, tag="sel")
        wtmp = const.tile([128, 16], f32, tag="wtmp")
        qt_sb = const.tile([128, PAIRS, S], bf16, tag="qt")  # rows 0-63 head A
        kt_sb = const.tile([128, PAIRS, S], bf16, tag="kt")
        vv = const.tile([128, VT_N, 4 * 65], bf16, tag="vv")  # [key,tt,(h,d|1)]
        at = const.tile([128, PAIRS, S], bf16, tag="at")  # pair-stacked a^T

        # ---- warm-up collectives: absorb launch skew off the critical path
        warm_in = dram.tile([128, 16], f32, tag="warm_in", name="warm_in")
        warm_out = dram.tile([128, 16], f32, tag="warm_out", name="warm_out")
        nc.vector.memset(wtmp[:], 0.0)
        nc.sync.dma_start(out=warm_in[:], in_=wtmp[:])
        for _ in range(2):
            nc.gpsimd.collective_compute(
                "AllReduce",
                mybir.AluOpType.add,
                replica_groups=[[0, 1, 2, 3], [4, 5, 6, 7]],
                ins=[warm_in[:].opt()],
                outs=[warm_out[:].opt()],
            )

        # ---- input DMAs (V path first; xT half 0 before half 1) ----
        nc.sync.dma_start(out=wv[:], in_=wv_d[:])
        nc.sync.dma_start(out=bv[:], in_=bv_d[:])
        nc.sync.dma_start(out=wq[:], in_=wq_d[:])
        nc.sync.dma_start(out=wk[:], in_=wk_d[:])
        nc.sync.dma_start(out=bq[:], in_=bq_d[:])
        nc.sync.dma_start(out=bk[:], in_=bk_d[:])
        nc.sync.dma_start(out=tri[:], in_=tri_d[:])
        nc.sync.dma_start(out=wp[:], in_=wp_d[:])
        nc.sync.dma_start(out=sel[:], in_=sel_d[:])
        for hf in range(2):
            for et in range(ET):
                nc.sync.dma_start(
                    out=xT[:, et, hf * 1024 : (hf + 1) * 1024],
                    in_=xT_d[et * 128 : (et + 1) * 128, hf * 1024 : (hf + 1) * 1024],
                )
        nc.vector.memset(vv.rearrange("p t (h e) -> p t h e", h=4)[:, :, :, 64:65], 1.0)

        # ---- Phase A: QKV projections (V first: AV needs it earliest) ----
        for tt in range(VT_N):
            sl = slice(tt * 128, (tt + 1) * 128)
            ps_v = psum_sc.tile([128, 1024], f32, tag="sc", name=f"ps_v{tt}")
            for et in range(ET):
                nc.tensor.matmul(
                    ps_v[:, 0:256],
                    lhsT=xT[:, et, sl],
                    rhs=wv[:, et],
                    start=(et == 0),
                    stop=(et == ET - 1),
                )
            nc.vector.tensor_tensor(
                out=vv.rearrange("p t (h e) -> p t h e", h=4)[:, tt, :, 0:64],
                in0=ps_v[:, 0:256].rearrange("p (h e) -> p h e", h=4),
                in1=bv.rearrange("p (h e) -> p h e", h=4),
                op=ALU.add,
            )
        for p in range(PAIRS):
            for tt in range(TT_N):
                sl = slice(tt * 512, (tt + 1) * 512)
                ps_qk = psum_sc.tile([128, 1024], f32, tag="sc", name=f"ps_qk{p}_{tt}")
                for et in range(ET):
                    nc.tensor.matmul(
                        ps_qk[:, 0:512],
                        lhsT=wq[:, p, et],
                        rhs=xT[:, et, sl],
                        start=(et == 0),
                        stop=(et == ET - 1),
                    )
                for et in range(ET):
                    nc.tensor.matmul(
                        ps_qk[:, 512:1024],
                        lhsT=wk[:, p, et],
                        rhs=xT[:, et, sl],
                        start=(et == 0),
                        stop=(et == ET - 1),
                    )
                nc.vector.tensor_scalar_add(
                    qt_sb[:, p, sl], ps_qk[:, 0:512], bq[:, p : p + 1]
                )
                nc.vector.tensor_scalar_add(
                    kt_sb[:, p, sl], ps_qk[:, 512:1024], bk[:, p : p + 1]
                )

        # ---- Phase B+C fused: per query-tile attention -> c_proj -> RS ----
        diag_c0 = (0, 128, 256, 384)
        cc_in = []
        cc_out = []
        for qt in range(QT_N):
            cc_in.append(
                dram.tile([512, 1024], bf16, tag=f"cc_in{qt}", name=f"cc_in{qt}")
            )
            cc_out.append(
                dram.tile([128, 1024], bf16, tag=f"cc_out{qt}", name=f"cc_out{qt}")
            )

        def flush_head(qt, den4, atu):
            """normalize (recip -> sel-matmul broadcast -> mult) for qt."""
            rec4 = pbuf.tile([128, 512], bf16, tag="recb", bufs=2, name=f"rec_{qt}")
            nc.vector.reciprocal(rec4[:], den4[:])
            for pi in range(PAIRS):
                rb = psum_cc.tile(
                    [128, 512], f32, tag="cc", bufs=2, name=f"rb_{qt}_{pi}"
                )
                nc.tensor.matmul(
                    rb, lhsT=sel[:, pi, :], rhs=rec4[:], start=True, stop=True
                )
                nc.vector.tensor_tensor(
                    out=at[:, pi, qt * 512 : (qt + 1) * 512],
                    in0=atu[pi][:],
                    in1=rb[:],
                    op=ALU.mult,
                )

        def flush_cproj(qt, i):
            """c_proj partial group i (of 8) for query tile qt -> stage DMA."""
            tt = 4 * qt + i // 2
            nt = i % 2
            ps_c = psum_cc.tile([128, 512], f32, tag="cc", bufs=2,
                                name=f"ps_c_{qt}_{i}")
            for pi in range(PAIRS):
                nc.tensor.matmul(
                    ps_c,
                    lhsT=at[:, pi, tt * 128 : (tt + 1) * 128],
                    rhs=wp[:, pi, nt * 512 : (nt + 1) * 512],
                    start=(pi == 0),
                    stop=(pi == PAIRS - 1),
                )
            cst = pbuf.tile([128, 512], bf16, tag="cstage", bufs=6,
                            name=f"cst_{qt}_{i}")
            nc.vector.tensor_copy(out=cst[:], in_=ps_c[:])
            nc.sync.dma_start(
                out=cc_in[qt][
                    (i // 2) * 128 : (i // 2 + 1) * 128, nt * 512 : (nt + 1) * 512
                ],
                in_=cst[:],
            )

        def flush_rs(qt):
            nc.gpsimd.collective_compute(
                "ReduceScatter",
                mybir.AluOpType.add,
                replica_groups=[[0, 1, 2, 3], [4, 5, 6, 7]],
                ins=[cc_in[qt][:].opt()],
                outs=[cc_out[qt][:].opt()],
            )
            nc.sync.dma_start(
                out=out_d[qt * 128 : (qt + 1) * 128, :], in_=cc_out[qt][:]
            )

        pending = None
        for qt in range(QT_N):
            if pending is not None:
                flush_head(*pending)
            den4 = pbuf.tile([128, 512], f32, tag="den", bufs=2, name=f"den_{qt}")
            nc.vector.memset(den4[:], 1.0)
            atu_pair = []
            ki = 0  # kt-iteration counter: interleave prev qt's c_proj groups
            for p in range(PAIRS):
                av = []
                for hh in range(2):
                    av.append(
                        psum_av.tile([65, 512], f32, tag="av", name=f"av_{p}_{qt}_{hh}")
                    )
                nkt = 4 * qt + 4
                for kt in range(nkt):
                    j = kt - 4 * qt
                    c0 = diag_c0[j] if j >= 0 else 0
                    n = 512 - c0
                    ps_s = psum_sc.tile([128, 1024], f32, tag="sc",
                                        name=f"ps_s_{p}_{qt}_{kt}")
                    for hh in range(2):
                        base = hh * 64
                        nc.tensor.matmul(
                            ps_s[:, hh * 512 + c0 : (hh + 1) * 512],
                            lhsT=kt_sb[base : base + 64, p, kt * 128 : (kt + 1) * 128],
                            rhs=qt_sb[base : base + 64, p, qt * 512 + c0 : (qt + 1) * 512],
                            start=True,
                            stop=True,
                        )
                    pt = pbuf.tile([128, 1024], bf16, tag="p", bufs=5,
                                   name=f"pt_{p}_{qt}_{kt}")
                    if j < 0:
                        nc.scalar.activation(pt[:], ps_s[:], AF.Exp)
                    else:
                        for hh in range(2):
                            nc.scalar.activation(
                                pt[:, hh * 512 : hh * 512 + n],
                                ps_s[:, hh * 512 + c0 : (hh + 1) * 512],
                                AF.Exp,
                            )
                        for hh in range(2):
                            nc.vector.tensor_tensor(
                                out=pt[:, hh * 512 : hh * 512 + 128],
                                in0=pt[:, hh * 512 : hh * 512 + 128],
                                in1=tri[:],
                                op=ALU.mult,
                            )
                    for hh in range(2):
                        h_idx = 2 * p + hh
                        nc.tensor.matmul(
                            av[hh][:, c0:512],
                            lhsT=vv[:, kt, h_idx * 65 : (h_idx + 1) * 65],
                            rhs=pt[:, hh * 512 : hh * 512 + n],
                            start=(kt == 0),
                            stop=(kt == nkt - 1),
                        )
                    if pending is not None and ki < 8:
                        flush_cproj(pending[0], ki)
                        if ki == 7:
                            flush_rs(pending[0])
                    ki += 1
                for hh in range(2):
                    h_idx = 2 * p + hh
                    nc.vector.tensor_copy(
                        out=den4[h_idx * 32 : h_idx * 32 + 1, :], in_=av[hh][64:65, :]
                    )
                atu2 = pbuf.tile([128, 512], f32, tag="atu", bufs=4,
                                 name=f"atu_{p}_{qt}")
                nc.vector.tensor_copy(out=atu2[0:64, :], in_=av[0][0:64, :])
                nc.vector.tensor_copy(out=atu2[64:128, :], in_=av[1][0:64, :])
                atu_pair.append(atu2)
            pending = (qt, den4, atu_pair)
        # tail: last query tile's normalize + c_proj + RS
        flush_head(*pending)
        for i in range(8):
            flush_cproj(QT_N - 1, i)
        flush_rs(QT_N - 1)

    nc.compile()
    return nc


def _prepare_in_maps(x, w_attn, b_attn, w_proj):
    import ml_dtypes

    bf = ml_dtypes.bfloat16
    in_maps = []
    tri = np.triu(np.ones((128, 128), dtype=bf))
    for core in range(N_CORES):
        b, g = core // 4, core % 4
        heads = [4 * g + i for i in range(4)]
        xT = np.ascontiguousarray(x[b].T)  # [1024, 2048]
        wq_blocks, wk_blocks, bq_cols, bk_cols = [], [], [], []
        for pr in range(PAIRS):
            hA, hB = heads[2 * pr], heads[2 * pr + 1]
            wq_blk = np.concatenate(
                [w_attn[:, hA * 192 : hA * 192 + 64], w_attn[:, hB * 192 : hB * 192 + 64]],
                axis=1,
            ) * 0.125
            wk_blk = np.concatenate(
                [
                    w_attn[:, hA * 192 + 64 : hA * 192 + 128],
                    w_attn[:, hB * 192 + 64 : hB * 192 + 128],
                ],
                axis=1,
            )
            # [1024,128] -> [128part, 8et, 128]
            wq_blocks.append(wq_blk.reshape(ET, 128, 128).transpose(1, 0, 2))
            wk_blocks.append(wk_blk.reshape(ET, 128, 128).transpose(1, 0, 2))
            bq_cols.append(
                np.concatenate(
                    [b_attn[hA * 192 : hA * 192 + 64], b_attn[hB * 192 : hB * 192 + 64]]
                ) * 0.125
            )
            bk_cols.append(
                np.concatenate(
                    [
                        b_attn[hA * 192 + 64 : hA * 192 + 128],
                        b_attn[hB * 192 + 64 : hB * 192 + 128],
                    ]
                )
            )
        wq_h = np.stack(wq_blocks, axis=1)  # [128, 2, 8, 128]
        wk_h = np.stack(wk_blocks, axis=1)
        wv_blk = np.concatenate(
            [w_attn[:, h * 192 + 128 : h * 192 + 192] for h in heads], axis=1
        )  # [1024, 256]
        wv_h = wv_blk.reshape(ET, 128, 256).transpose(1, 0, 2)  # [128, 8, 256]
        bv_row = np.concatenate(
            [b_attn[h * 192 + 128 : h * 192 + 192] for h in heads]
        )  # [256]
        bv_h = np.broadcast_to(bv_row, (128, 256)).copy()
        wp_h = np.empty((128, PAIRS, 1024), dtype=np.float32)
        sel_h = np.zeros((128, PAIRS, 128), dtype=np.float32)
        for pr in range(PAIRS):
            hA, hB = heads[2 * pr], heads[2 * pr + 1]
            wp_h[0:64, pr, :] = w_proj[hA * 64 : (hA + 1) * 64, :]
            wp_h[64:128, pr, :] = w_proj[hB * 64 : (hB + 1) * 64, :]
            sel_h[(2 * pr) * 32, pr, 0:64] = 1.0
            sel_h[(2 * pr + 1) * 32, pr, 64:128] = 1.0
        in_maps.append(
            {
                "xT": np.ascontiguousarray(xT.astype(bf)),
                "wq": np.ascontiguousarray(wq_h.astype(bf)),
                "wk": np.ascontiguousarray(wk_h.astype(bf)),
                "wv": np.ascontiguousarray(wv_h.astype(bf)),
                "bq": np.ascontiguousarray(np.stack(bq_cols, 1), dtype=np.float32),
                "bk": np.ascontiguousarray(np.stack(bk_cols, 1), dtype=np.float32),
                "bv": bv_h.astype(np.float32),
                "tri": tri,
                "wp": np.ascontiguousarray(wp_h.astype(bf)),
                "sel": np.ascontiguousarray(sel_h.astype(bf)),
            }
        )
    return in_maps


def _run(x, w_attn, b_attn, w_proj, b_proj, trace=False):
    from concourse.bass_utils import run_bass_kernel_spmd

    if "nc" not in _cache:
        _cache["nc"] = _build()
    nc = _cache["nc"]
    in_maps = _prepare_in_maps(x, w_attn, b_attn, w_proj)
    res = run_bass_kernel_spmd(nc, in_maps, list(range(N_CORES)), trace=trace)
    outs = []
    for b in range(B):
        full = np.empty((S, E), dtype=np.float32)
        for r_ in range(4):
            core_out = res.results[4 * b + r_]["out"]
            for qt in range(QT_N):
                full[qt * 512 + r_ * 128 : qt * 512 + (r_ + 1) * 128] = core_out[
                    qt * 128 : (qt + 1) * 128
                ]
        outs.append(full + b_proj[None, :])
    return np.stack(outs).astype(np.float32), res


def kernel(x, w_attn, b_attn, w_proj, b_proj):
    x = np.asarray(x, dtype=np.float32)
    w_attn = np.asarray(w_attn, dtype=np.float32)
    b_attn = np.asarray(b_attn, dtype=np.float32)
    w_proj = np.asarray(w_proj, dtype=np.float32)
    b_proj = np.asarray(b_proj, dtype=np.float32)
    out, _ = _run(x, w_attn, b_attn, w_proj, b_proj, trace=False)
    return out


# revision 7
# speedup vs baseline: 1.3874x; 1.0086x over previous
"""Trainium2 Bass kernel for causal multi-head attention block (GPT-style).

Reference computation (fp32):
    qkv = x @ w_attn + b_attn          # [B,S,3E], heads interleaved per 192 cols
    q,k,v per head (d=64), scores = q k^T / 8, causal mask, softmax
    a = softmax @ v ; h = a @ w_proj + b_proj

Sharding (8 cores): core c -> batch b = c//4, head group g = c%4 (4 heads).
Each core computes qkv for its heads, full causal attention, and a partial
c_proj over its 256 e_in rows; a 4-way ReduceScatter(add) per batch group
(bf16 wire) yields each core's token chunks of the final output. b_proj on
host.

Query tiling: groups of 512,512,512,256,256 tokens. The last 512 is split
so the final ReduceScatter (the serial tail) is half-sized and the
second-to-last one hides under the last group's attention.

Pipelining: two tiny warm-up AllReduces absorb core launch skew during the
QKV phase; xT streams per 512-token group with QKV compute interleaved;
the c_proj + staging of the previous query group is interleaved into the
first kt iterations of the next group's attention so PE/ACT never drain.
"""

import os
import sys

import numpy as np

if "/opt/trn_rl_repo" not in sys.path:
    sys.path.insert(0, "/opt/trn_rl_repo")

B, S, E, H, D = 2, 2048, 1024, 16, 64
N_CORES = 8
PAIRS = 2  # head pairs per core
ET = 8  # e tiles of 128 over E=1024
TT_N = 4  # token tiles of 512 (qkv QK rhs)
VT_N = 16  # token tiles of 128 (V / c_proj)
QGROUPS = ((0, 512), (512, 512), (1024, 512), (1536, 256), (1792, 256))

_cache = {}


def _build():
    import concourse.bass as bass
    import concourse.mybir as mybir
    import concourse.tile as tile
    from concourse import bacc
    from contextlib import ExitStack

    f32 = mybir.dt.float32
    bf16 = mybir.dt.bfloat16
    ALU = mybir.AluOpType
    AF = mybir.ActivationFunctionType

    nc = bacc.Bacc(
        "TRN2", target_bir_lowering=False, debug=False, num_devices=N_CORES
    )

    xT_d = nc.declare_dram_parameter("xT", [E, S], bf16, isOutput=False)
    wq_d = nc.declare_dram_parameter("wq", [128, PAIRS, ET, 128], bf16, isOutput=False)
    wk_d = nc.declare_dram_parameter("wk", [128, PAIRS, ET, 128], bf16, isOutput=False)
    wv_d = nc.declare_dram_parameter("wv", [128, ET, 256], bf16, isOutput=False)
    bq_d = nc.declare_dram_parameter("bq", [128, PAIRS], f32, isOutput=False)
    bk_d = nc.declare_dram_parameter("bk", [128, PAIRS], f32, isOutput=False)
    bv_d = nc.declare_dram_parameter("bv", [128, 256], f32, isOutput=False)
    tri_d = nc.declare_dram_parameter("tri", [128, 128], bf16, isOutput=False)
    wp_d = nc.declare_dram_parameter("wp", [128, PAIRS, 1024], bf16, isOutput=False)
    sel_d = nc.declare_dram_parameter("sel", [128, PAIRS, 128], bf16, isOutput=False)
    out_d = nc.declare_dram_parameter("out", [512, 1024], bf16, isOutput=True)

    with ExitStack() as ctx:
        ctx.enter_context(
            nc.allow_low_precision(reason="bf16 matmuls/collectives within 2e-2 tol")
        )
        tc = ctx.enter_context(tile.TileContext(nc))
        const = ctx.enter_context(tc.tile_pool(name="const", bufs=1))
        dram = ctx.enter_context(tc.tile_pool(name="dram", bufs=1, space="DRAM"))
        psum_sc = ctx.enter_context(tc.tile_pool(name="psum_sc", bufs=2, space="PSUM"))
        psum_av = ctx.enter_context(tc.tile_pool(name="psum_av", bufs=2, space="PSUM"))
        psum_cc = ctx.enter_context(tc.tile_pool(name="psum_cc", bufs=2, space="PSUM"))
        pbuf = ctx.enter_context(tc.tile_pool(name="pbuf", bufs=6))

        # ---- persistent SBUF tensors ----
        xT = const.tile([128, ET, S], bf16, tag="xT")  # 4 MB
        wq = const.tile([128, PAIRS, ET, 128], bf16, tag="wq")
        wk = const.tile([128, PAIRS, ET, 128], bf16, tag="wk")
        wv = const.tile([128, ET, 256], bf16, tag="wv")
        bq = const.tile([128, PAIRS], f32, tag="bq")
        bk = const.tile([128, PAIRS], f32, tag="bk")
        bv = const.tile([128, 256], f32, tag="bv")
        tri = const.tile([128, 128], bf16, tag="tri")
        wp = const.tile([128, PAIRS, 1024], bf16, tag="wp")
        sel = const.tile([128, PAIRS, 128], bf16, tag="sel")
        wtmp = const.tile([128, 16], f32, tag="wtmp")
        qt_sb = const.tile([128, PAIRS, S], bf16, tag="qt")  # rows 0-63 head A
        kt_sb = const.tile([128, PAIRS, S], bf16, tag="kt")
        vv = const.tile([128, VT_N, 4 * 65], bf16, tag="vv")  # [key,tt,(h,d|1)]
        at = const.tile([128, PAIRS, S], bf16, tag="at")  # pair-stacked a^T

        # ---- warm-up collectives: absorb launch skew off the critical path
        warm_in = dram.tile([128, 16], f32, tag="warm_in", name="warm_in")
        warm_out = dram.tile([128, 16], f32, tag="warm_out", name="warm_out")
        nc.vector.memset(wtmp[:], 0.0)
        nc.sync.dma_start(out=warm_in[:], in_=wtmp[:])
        for _ in range(2):
            nc.gpsimd.collective_compute(
                "AllReduce",
                mybir.AluOpType.add,
                replica_groups=[[0, 1, 2, 3], [4, 5, 6, 7]],
                ins=[warm_in[:].opt()],
                outs=[warm_out[:].opt()],
            )

        # ---- input DMAs (Q/K weights first, then xT streamed per group) ----
        nc.sync.dma_start(out=wq[:], in_=wq_d[:])
        nc.sync.dma_start(out=wk[:], in_=wk_d[:])
        nc.sync.dma_start(out=bq[:], in_=bq_d[:])
        nc.sync.dma_start(out=bk[:], in_=bk_d[:])
        nc.sync.dma_start(out=wv[:], in_=wv_d[:])
        nc.sync.dma_start(out=bv[:], in_=bv_d[:])
        for grp in range(TT_N):
            for et in range(ET):
                nc.sync.dma_start(
                    out=xT[:, et, grp * 512 : (grp + 1) * 512],
                    in_=xT_d[et * 128 : (et + 1) * 128, grp * 512 : (grp + 1) * 512],
                )
        nc.sync.dma_start(out=tri[:], in_=tri_d[:])
        nc.sync.dma_start(out=wp[:], in_=wp_d[:])
        nc.sync.dma_start(out=sel[:], in_=sel_d[:])
        nc.vector.memset(vv.rearrange("p t (h e) -> p t h e", h=4)[:, :, :, 64:65], 1.0)

        # ---- Phase A: QKV projections, interleaved per 512-token group ----
        for grp in range(TT_N):
            sl = slice(grp * 512, (grp + 1) * 512)
            for p in range(PAIRS):
                ps_qk = psum_sc.tile(
                    [128, 1024], f32, tag="sc", name=f"ps_qk{p}_{grp}"
                )
                for et in range(ET):
                    nc.tensor.matmul(
                        ps_qk[:, 0:512],
                        lhsT=wq[:, p, et],
                        rhs=xT[:, et, sl],
                        start=(et == 0),
                        stop=(et == ET - 1),
                    )
                for et in range(ET):
                    nc.tensor.matmul(
                        ps_qk[:, 512:1024],
                        lhsT=wk[:, p, et],
                        rhs=xT[:, et, sl],
                        start=(et == 0),
                        stop=(et == ET - 1),
                    )
                nc.vector.tensor_scalar_add(
                    qt_sb[:, p, sl], ps_qk[:, 0:512], bq[:, p : p + 1]
                )
                nc.vector.tensor_scalar_add(
                    kt_sb[:, p, sl], ps_qk[:, 512:1024], bk[:, p : p + 1]
                )
            for tt in range(4 * grp, 4 * grp + 4):
                vsl = slice(tt * 128, (tt + 1) * 128)
                ps_v = psum_sc.tile([128, 1024], f32, tag="sc", name=f"ps_v{tt}")
                for et in range(ET):
                    nc.tensor.matmul(
                        ps_v[:, 0:256],
                        lhsT=xT[:, et, vsl],
                        rhs=wv[:, et],
                        start=(et == 0),
                        stop=(et == ET - 1),
                    )
                nc.vector.tensor_tensor(
                    out=vv.rearrange("p t (h e) -> p t h e", h=4)[:, tt, :, 0:64],
                    in0=ps_v[:, 0:256].rearrange("p (h e) -> p h e", h=4),
                    in1=bv.rearrange("p (h e) -> p h e", h=4),
                    op=ALU.add,
                )

        # ---- Phase B+C fused: per query-group attention -> c_proj -> RS ----
        cc_in = []
        cc_out = []
        for g, (q0, qw) in enumerate(QGROUPS):
            cc_in.append(
                dram.tile([qw, 1024], bf16, tag=f"cc_in{g}", name=f"cc_in{g}")
            )
            cc_out.append(
                dram.tile([qw // 4, 1024], bf16, tag=f"cc_out{g}", name=f"cc_out{g}")
            )

        def flush_head(g, den4, atu):
            """normalize (recip -> sel-matmul broadcast -> mult) for group g."""
            q0, qw = QGROUPS[g]
            rec_f = pbuf.tile([128, 512], f32, tag="recf", bufs=2, name=f"recf_{g}")
            rec4 = pbuf.tile([128, 512], bf16, tag="recb", bufs=2, name=f"rec_{g}")
            nc.vector.reciprocal_approx_fast(rec_f[:, 0:qw], den4[:, 0:qw])
            nc.vector.tensor_copy(out=rec4[:, 0:qw], in_=rec_f[:, 0:qw])
            for pi in range(PAIRS):
                rb = psum_cc.tile(
                    [128, 512], f32, tag="cc", bufs=2, name=f"rb_{g}_{pi}"
                )
                nc.tensor.matmul(
                    rb[:, 0:qw], lhsT=sel[:, pi, :], rhs=rec4[:, 0:qw],
                    start=True, stop=True,
                )
                nc.vector.tensor_tensor(
                    out=at[:, pi, q0 : q0 + qw],
                    in0=atu[pi][:, 0:qw],
                    in1=rb[:, 0:qw],
                    op=ALU.mult,
                )

        def flush_cproj(g, i):
            """c_proj partial group i (of qw//64) for query group g."""
            q0, qw = QGROUPS[g]
            tt = q0 // 128 + i // 2
            nt = i % 2
            ps_c = psum_cc.tile([128, 512], f32, tag="cc", bufs=2,
                                name=f"ps_c_{g}_{i}")
            for pi in range(PAIRS):
                nc.tensor.matmul(
                    ps_c,
                    lhsT=at[:, pi, tt * 128 : (tt + 1) * 128],
                    rhs=wp[:, pi, nt * 512 : (nt + 1) * 512],
                    start=(pi == 0),
                    stop=(pi == PAIRS - 1),
                )
            cst = pbuf.tile([128, 512], bf16, tag="cstage", bufs=6,
                            name=f"cst_{g}_{i}")
            nc.vector.tensor_copy(out=cst[:], in_=ps_c[:])
            nc.sync.dma_start(
                out=cc_in[g][
                    (i // 2) * 128 : (i // 2 + 1) * 128, nt * 512 : (nt + 1) * 512
                ],
                in_=cst[:],
            )

        def flush_rs(g):
            q0, qw = QGROUPS[g]
            nc.gpsimd.collective_compute(
                "ReduceScatter",
                mybir.AluOpType.add,
                replica_groups=[[0, 1, 2, 3], [4, 5, 6, 7]],
                ins=[cc_in[g][:].opt()],
                outs=[cc_out[g][:].opt()],
            )
            nc.sync.dma_start(
                out=out_d[q0 // 4 : (q0 + qw) // 4, :], in_=cc_out[g][:]
            )

        pending = None
        for g, (q0, qw) in enumerate(QGROUPS):
            if pending is not None:
                flush_head(pending[0], pending[1], pending[2])
            den4 = pbuf.tile([128, 512], f32, tag="den", bufs=2, name=f"den_{g}")
            nc.vector.memset(den4[:], 1.0)
            atu_pair = []
            nflush = 0 if pending is None else QGROUPS[pending[0]][1] // 64
            ki = 0  # kt-iteration counter: interleave prev group's c_proj
            for p in range(PAIRS):
                av = []
                for hh in range(2):
                    av.append(
                        psum_av.tile([65, 512], f32, tag="av", name=f"av_{p}_{g}_{hh}")
                    )
                nkt = (q0 + qw) // 128
                for kt in range(nkt):
                    c0 = max(0, kt * 128 - q0)
                    n = qw - c0
                    diag = kt * 128 >= q0
                    ps_s = psum_sc.tile([128, 1024], f32, tag="sc",
                                        name=f"ps_s_{p}_{g}_{kt}")
                    for hh in range(2):
                        base = hh * 64
                        nc.tensor.matmul(
                            ps_s[:, hh * 512 + c0 : hh * 512 + qw],
                            lhsT=kt_sb[base : base + 64, p, kt * 128 : (kt + 1) * 128],
                            rhs=qt_sb[base : base + 64, p, q0 + c0 : q0 + qw],
                            start=True,
                            stop=True,
                        )
                    pt = pbuf.tile([128, 1024], bf16, tag="p", bufs=5,
                                   name=f"pt_{p}_{g}_{kt}")
                    if not diag:
                        if qw == 512:
                            nc.scalar.activation(pt[:], ps_s[:], AF.Exp)
                        else:
                            for hh in range(2):
                                nc.scalar.activation(
                                    pt[:, hh * 512 : hh * 512 + qw],
                                    ps_s[:, hh * 512 : hh * 512 + qw],
                                    AF.Exp,
                                )
                    else:
                        for hh in range(2):
                            nc.scalar.activation(
                                pt[:, hh * 512 : hh * 512 + n],
                                ps_s[:, hh * 512 + c0 : hh * 512 + qw],
                                AF.Exp,
                            )
                        for hh in range(2):
                            nc.vector.tensor_tensor(
                                out=pt[:, hh * 512 : hh * 512 + 128],
                                in0=pt[:, hh * 512 : hh * 512 + 128],
                                in1=tri[:],
                                op=ALU.mult,
                            )
                    for hh in range(2):
                        h_idx = 2 * p + hh
                        nc.tensor.matmul(
                            av[hh][:, c0:qw],
                            lhsT=vv[:, kt, h_idx * 65 : (h_idx + 1) * 65],
                            rhs=pt[:, hh * 512 : hh * 512 + n],
                            start=(kt == 0),
                            stop=(kt == nkt - 1),
                        )
                    if ki < nflush:
                        flush_cproj(pending[0], ki)
                        if ki == nflush - 1:
                            flush_rs(pending[0])
                    ki += 1
                for hh in range(2):
                    h_idx = 2 * p + hh
                    nc.vector.tensor_copy(
                        out=den4[h_idx * 32 : h_idx * 32 + 1, 0:qw],
                        in_=av[hh][64:65, 0:qw],
                    )
                atu2 = pbuf.tile([128, 512], f32, tag="atu", bufs=4,
                                 name=f"atu_{p}_{g}")
                nc.scalar.copy(atu2[0:64, 0:qw], av[0][0:64, 0:qw])
                nc.scalar.copy(atu2[64:128, 0:qw], av[1][0:64, 0:qw])
                atu_pair.append(atu2)
            pending = (g, den4, atu_pair)
        # tail: last query group's normalize + c_proj + RS (half-sized)
        flush_head(pending[0], pending[1], pending[2])
        for i in range(QGROUPS[pending[0]][1] // 64):
            flush_cproj(pending[0], i)
        flush_rs(pending[0])

    nc.compile()
    return nc


def _prepare_in_maps(x, w_attn, b_attn, w_proj):
    import ml_dtypes

    bf = ml_dtypes.bfloat16
    in_maps = []
    tri = np.triu(np.ones((128, 128), dtype=bf))
    for core in range(N_CORES):
        b, g = core // 4, core % 4
        heads = [4 * g + i for i in range(4)]
        xT = np.ascontiguousarray(x[b].T)  # [1024, 2048]
        wq_blocks, wk_blocks, bq_cols, bk_cols = [], [], [], []
        for pr in range(PAIRS):
            hA, hB = heads[2 * pr], heads[2 * pr + 1]
            wq_blk = np.concatenate(
                [w_attn[:, hA * 192 : hA * 192 + 64], w_attn[:, hB * 192 : hB * 192 + 64]],
                axis=1,
            ) * 0.125
            wk_blk = np.concatenate(
                [
                    w_attn[:, hA * 192 + 64 : hA * 192 + 128],
                    w_attn[:, hB * 192 + 64 : hB * 192 + 128],
                ],
                axis=1,
            )
            # [1024,128] -> [128part, 8et, 128]
            wq_blocks.append(wq_blk.reshape(ET, 128, 128).transpose(1, 0, 2))
            wk_blocks.append(wk_blk.reshape(ET, 128, 128).transpose(1, 0, 2))
            bq_cols.append(
                np.concatenate(
                    [b_attn[hA * 192 : hA * 192 + 64], b_attn[hB * 192 : hB * 192 + 64]]
                ) * 0.125
            )
            bk_cols.append(
                np.concatenate(
                    [
                        b_attn[hA * 192 + 64 : hA * 192 + 128],
                        b_attn[hB * 192 + 64 : hB * 192 + 128],
                    ]
                )
            )
        wq_h = np.stack(wq_blocks, axis=1)  # [128, 2, 8, 128]
        wk_h = np.stack(wk_blocks, axis=1)
        wv_blk = np.concatenate(
            [w_attn[:, h * 192 + 128 : h * 192 + 192] for h in heads], axis=1
        )  # [1024, 256]
        wv_h = wv_blk.reshape(ET, 128, 256).transpose(1, 0, 2)  # [128, 8, 256]
        bv_row = np.concatenate(
            [b_attn[h * 192 + 128 : h * 192 + 192] for h in heads]
        )  # [256]
        bv_h = np.broadcast_to(bv_row, (128, 256)).copy()
        wp_h = np.empty((128, PAIRS, 1024), dtype=np.float32)
        sel_h = np.zeros((128, PAIRS, 128), dtype=np.float32)
        for pr in range(PAIRS):
            hA, hB = heads[2 * pr], heads[2 * pr + 1]
            wp_h[0:64, pr, :] = w_proj[hA * 64 : (hA + 1) * 64, :]
            wp_h[64:128, pr, :] = w_proj[hB * 64 : (hB + 1) * 64, :]
            sel_h[(2 * pr) * 32, pr, 0:64] = 1.0
            sel_h[(2 * pr + 1) * 32, pr, 64:128] = 1.0
        in_maps.append(
            {
                "xT": np.ascontiguousarray(xT.astype(bf)),
                "wq": np.ascontiguousarray(wq_h.astype(bf)),
                "wk": np.ascontiguousarray(wk_h.astype(bf)),
                "wv": np.ascontiguousarray(wv_h.astype(bf)),
                "bq": np.ascontiguousarray(np.stack(bq_cols, 1), dtype=np.float32),
                "bk": np.ascontiguousarray(np.stack(bk_cols, 1), dtype=np.float32),
                "bv": bv_h.astype(np.float32),
                "tri": tri,
                "wp": np.ascontiguousarray(wp_h.astype(bf)),
                "sel": np.ascontiguousarray(sel_h.astype(bf)),
            }
        )
    return in_maps


def _run(x, w_attn, b_attn, w_proj, b_proj, trace=False):
    from concourse.bass_utils import run_bass_kernel_spmd

    if "nc" not in _cache:
        _cache["nc"] = _build()
    nc = _cache["nc"]
    in_maps = _prepare_in_maps(x, w_attn, b_attn, w_proj)
    res = run_bass_kernel_spmd(nc, in_maps, list(range(N_CORES)), trace=trace)
    outs = []
    for b in range(B):
        full = np.empty((S, E), dtype=np.float32)
        for r_ in range(4):
            core_out = res.results[4 * b + r_]["out"]
            for q0, qw in QGROUPS:
                c = qw // 4
                full[q0 + r_ * c : q0 + (r_ + 1) * c] = core_out[
                    q0 // 4 : q0 // 4 + c
                ]
        outs.append(full + b_proj[None, :])
    return np.stack(outs).astype(np.float32), res


def kernel(x, w_attn, b_attn, w_proj, b_proj):
    x = np.asarray(x, dtype=np.float32)
    w_attn = np.asarray(w_attn, dtype=np.float32)
    b_attn = np.asarray(b_attn, dtype=np.float32)
    w_proj = np.asarray(w_proj, dtype=np.float32)
    b_proj = np.asarray(b_proj, dtype=np.float32)
    out, _ = _run(x, w_attn, b_attn, w_proj, b_proj, trace=False)
    return out
